# revision 33
# baseline (speedup 1.0000x reference)
"""Multi-head attention (B=16, N=1024, E=768, H=12, D=64) on 8 TRN2 NeuronCores.

Strategy: data-parallel over batch (2 batches per core, no collectives).
Per-core kernel computes qkv = x @ w_qkv + b, per-head attention, out proj.

Layout choices (all compute in bf16, fp32 PSUM accumulation):
  - x^T (embed-on-partitions) built once per batch via PE transpose.
  - q^T/k^T produced feature-major [feat, tokens]: S^T = k^T.T @ q^T needs
    no transposes at all (keys on PSUM partitions, queries on free dim).
  - exp fused into the PSUM->SBUF copy on the scalar (ACT) engine; softmax
    denominator obtained free by appending an all-ones column to each
    v-tile in the P@V matmul (row 64 of the PV psum = sum of exp scores).
  - normalization: fast approx reciprocal of the sum row, gpsimd partition
    broadcast, one DVE multiply; odd heads partition-shifted into the
    emb-major O^T store via a small SBUF->SBUF DMA.
  - out = O^T.T @ w_proj directly token-major for a contiguous DRAM store.

Scheduling: attention is ACT(exp)-bound per head, so PE work from other
phases is interleaved as "fillers" — batch b+1's x^T/QKV projections run
inside batch b's attention, batch b-1's out-projection inside batch b's —
keeping TensorE dense (HAM stays un-throttled) while the scalar engine
streams the exps.
"""

from contextlib import ExitStack

import numpy as np

import concourse.bass as bass
import concourse.mybir as mybir
import concourse.tile as tile
from concourse import bacc
from concourse.bass_utils import run_bass_kernel_spmd
from concourse.masks import make_identity

F32 = mybir.dt.float32
BF16 = mybir.dt.bfloat16
AF = mybir.ActivationFunctionType
OP = mybir.AluOpType

P = 128
E = 768          # embed dim
H = 12           # heads
D = 64           # head dim
KO = E // P      # 6 contraction subtiles over embed
B_FULL = 16
N_FULL = 1024
N_CORES = 8
BPC = B_FULL // N_CORES  # batches per core


def _body(ctx, tc, out_d, x_d, wqkv_d, bqkv_d, wproj_d, bproj_d, n_batch, N):
    nc = tc.nc
    TT = N // P                 # token tiles per batch
    CS = min(512, N)            # query-chunk size
    NCH = N // CS               # query chunks

    const = ctx.enter_context(tc.tile_pool(name="const", bufs=1))
    xt_pool = ctx.enter_context(tc.tile_pool(name="xt", bufs=1))
    qk_pool = ctx.enter_context(tc.tile_pool(name="qk", bufs=2))
    v_pool = ctx.enter_context(tc.tile_pool(name="v", bufs=1))
    pt_pool = ctx.enter_context(tc.tile_pool(name="pt", bufs=2))
    ot_pool = ctx.enter_context(tc.tile_pool(name="ot", bufs=1))
    small = ctx.enter_context(tc.tile_pool(name="small", bufs=3))
    wstage = ctx.enter_context(tc.tile_pool(name="wstage", bufs=3))
    xstage = ctx.enter_context(tc.tile_pool(name="xstage", bufs=2))
    nrm = ctx.enter_context(tc.tile_pool(name="nrm", bufs=1))
    nrm2 = ctx.enter_context(tc.tile_pool(name="nrm2", bufs=2))
    dram = ctx.enter_context(tc.tile_pool(name="dram", bufs=1, space="DRAM"))
    # [128, 768] f32 staging shared by weight loads, x loads, and out tiles
    osb_pool = ctx.enter_context(tc.tile_pool(name="osb", bufs=2))
    psA = ctx.enter_context(tc.tile_pool(name="psA", bufs=2, space="PSUM"))
    psB = ctx.enter_context(tc.tile_pool(name="psB", bufs=3, space="PSUM"))
    psO = ctx.enter_context(tc.tile_pool(name="psO", bufs=1, space="PSUM"))

    def sa_tile():
        return psA.tile([P, 1024], F32, tag="psa", name="psa")

    def sb_tile():
        return psB.tile([P, 512], F32, tag="psb", name="psb")

    def stage_tile():
        return osb_pool.tile([P, E], F32, tag="osb", name="osb")

    # ---------------- constants ----------------
    identity = const.tile([P, P], F32, tag="ident")
    make_identity(nc, identity)

    wqkv_sb = const.tile([P, KO, 3 * E], BF16, tag="wqkv")
    wproj_sb = const.tile([P, KO, E], BF16, tag="wproj")
    bqk_sb = const.tile([P, 2 * KO], F32, tag="bqk")
    bv_sb = const.tile([P, E], F32, tag="bv")
    bp_sb = const.tile([P, E], F32, tag="bp")
    wq_r = wqkv_d.rearrange("(ko p) (c n) -> p ko c n", p=P, n=E)
    wp_r = wproj_d.rearrange("(ko p) n -> p ko n", p=P)

    def bias_chunk():
        # q/k bias as per-partition columns: feature f = m*128+p -> [128,12]
        nc.sync.dma_start(
            bqk_sb[:], bqkv_d[: 2 * E].rearrange("(m p) -> p m", p=P)
        )
        # v / proj bias replicated across partitions (free-dim varying)
        nc.gpsimd.dma_start(
            bv_sb[:], bqkv_d[2 * E : 3 * E].partition_broadcast(P)
        )
        nc.gpsimd.dma_start(bp_sb[:], bproj_d.partition_broadcast(P))

    def wqkv_chunk(ko, c):
        def emit():
            t = wstage.tile([P, E], F32, tag="ws", name="ws")
            nc.sync.dma_start(t[:], wq_r[:, ko, c, :])
            nc.vector.tensor_copy(wqkv_sb[:, ko, c * E : (c + 1) * E], t[:])

        return emit

    def wproj_chunk(ko):
        def emit():
            t = wstage.tile([P, E], F32, tag="ws", name="ws")
            nc.sync.dma_start(t[:], wp_r[:, ko, :])
            nc.vector.tensor_copy(wproj_sb[:, ko, :], t[:])

        return emit

    # Per-batch state handles
    xT = [None] * n_batch
    qkT = [None] * n_batch
    v_sb = [None] * n_batch
    OT = [None] * n_batch

    # ---------------- phase emitters (closures) -----------------------
    def xprep_chunk(b, tt):
        def emit():
            if tt == 0:
                xT[b] = xt_pool.tile([P, KO, N], BF16, tag="xT", name="xT")
            xtmp = xstage.tile([P, E], F32, tag="xs", name="xs")
            nc.sync.dma_start(xtmp[:], x_d[b, tt * P : (tt + 1) * P, :])
            for ko in range(KO):
                pst = sb_tile()
                nc.tensor.transpose(
                    pst[:, :P], xtmp[:, ko * P : (ko + 1) * P], identity
                )
                nc.vector.tensor_copy(
                    xT[b][:, ko, tt * P : (tt + 1) * P], pst[:, :P]
                )

        return emit

    def qk_chunk(b, m, ch):
        def emit():
            if m == 0 and ch == 0:
                qkT[b] = qk_pool.tile([P, 2 * KO, N], BF16, tag="qkT", name="qkT")
            ps = sb_tile()
            for ko in range(KO):
                nc.tensor.matmul(
                    ps[:, :CS],
                    wqkv_sb[:, ko, m * P : (m + 1) * P],
                    xT[b][:, ko, ch * CS : (ch + 1) * CS],
                    start=(ko == 0),
                    stop=(ko == KO - 1),
                )
            nc.vector.tensor_tensor(
                qkT[b][:, m, ch * CS : (ch + 1) * CS],
                ps[:, :CS],
                bqk_sb[:, m : m + 1].to_broadcast([P, CS]),
                OP.add,
            )

        return emit

    def v_chunk(b, tt, oc):
        def emit():
            if tt == 0 and oc == 0:
                v_sb[b] = v_pool.tile([P, TT, H * 65], BF16, tag="v", name="v")
                ones_cols = v_sb[b].rearrange("p t (h c) -> p t h c", c=65)[
                    :, :, :, 64
                ]
                nc.vector.memset(ones_cols, 1.0)
            ocs = 512 if oc == 0 else 256
            nh = 8 if oc == 0 else 4
            ps = sb_tile()
            for ko in range(KO):
                nc.tensor.matmul(
                    ps[:, :ocs],
                    xT[b][:, ko, tt * P : (tt + 1) * P],
                    wqkv_sb[:, ko, 2 * E + oc * 512 : 2 * E + oc * 512 + ocs],
                    start=(ko == 0),
                    stop=(ko == KO - 1),
                )
            for hh in range(nh):
                h = oc * 8 + hh
                nc.vector.tensor_tensor(
                    v_sb[b][:, tt, h * 65 : h * 65 + 64],
                    ps[:, hh * 64 : (hh + 1) * 64],
                    bv_sb[:, h * 64 : (h + 1) * 64],
                    OP.add,
                )

        return emit

    _osb_state = {}

    def oproj_chunk(b, tt, oc):
        def emit():
            if oc == 0:
                _osb_state[(b, tt)] = stage_tile()
            osb = _osb_state[(b, tt)]
            ocs = 512 if oc == 0 else 256
            ps = sb_tile()
            for ko in range(KO):
                nc.tensor.matmul(
                    ps[:, :ocs],
                    OT[b][:, ko, tt * P : (tt + 1) * P],
                    wproj_sb[:, ko, oc * 512 : oc * 512 + ocs],
                    start=(ko == 0),
                    stop=(ko == KO - 1),
                )
            nc.vector.tensor_tensor(
                osb[:, oc * 512 : oc * 512 + ocs],
                ps[:, :ocs],
                bp_sb[:, oc * 512 : oc * 512 + ocs],
                OP.add,
            )
            if oc == 1:
                nc.sync.dma_start(
                    out_d[b, tt * P : (tt + 1) * P, :], osb[:]
                )

        return emit

    # ---------------- attention -----------------------
    def emit_scores(b, h, pop_filler, n_pops):
        ft, pr = h // 2, (h % 2) * 64
        pop_at = {3: 1, 7: 1} if n_pops <= 2 else {1: 1, 3: 1, 5: 1, 7: n_pops - 3}
        if n_pops == 1:
            pop_at = {7: 1}
        if n_pops == 0:
            pop_at = {}
        Pt = pt_pool.tile([P, TT, N], BF16, tag="Pt", name="Pt")
        for kt in range(TT):
            ps = sa_tile()
            for ch in range(NCH):
                nc.tensor.matmul(
                    ps[:, ch * CS : (ch + 1) * CS],
                    qkT[b][pr : pr + 64, KO + ft, kt * P : (kt + 1) * P],
                    qkT[b][pr : pr + 64, ft, ch * CS : (ch + 1) * CS],
                    start=True,
                    stop=True,
                )
            nc.scalar.activation(Pt[:, kt, :], ps[:, :N], AF.Exp, scale=0.125)
            for _ in range(pop_at.get(kt, 0)):
                pop_filler()
        return Pt

    def make_pv_pieces(b, h, Pt, sums):
        """PV matmuls + epilogue for head h as a list of closures, to be
        consumed interleaved with the NEXT head's score matmuls so the
        scalar engine never starves while PE runs the PV block."""
        ft, pr = h // 2, (h % 2) * 64
        pieces = []
        state = {}
        for ch in range(NCH):

            def mk_mm(ch, g):
                def emit():
                    if g == 0:
                        state[ch] = psO.tile([P, 512], F32, tag="po", name="po")
                    po = state[ch]
                    for kt in (2 * g, 2 * g + 1):
                        nc.tensor.matmul(
                            po[:65, :CS],
                            v_sb[b][:, kt, h * 65 : (h + 1) * 65],
                            Pt[:, kt, ch * CS : (ch + 1) * CS],
                            start=(kt == 0),
                            stop=(kt == TT - 1),
                        )

                return emit

            def mk_tail(ch):
                def emit():
                    po = state[ch]
                    otmp = small.tile([65, 512], BF16, tag="otmp", name="otmp")
                    nc.vector.tensor_copy(otmp[:, :CS], po[0:65, :CS])
                    nc.sync.dma_start(
                        sums[h, ch * CS : (ch + 1) * CS], otmp[64:65, :CS]
                    )
                    nc.sync.dma_start(
                        OT[b][pr : pr + 64, ft, ch * CS : (ch + 1) * CS],
                        otmp[0:64, :CS],
                    )

                return emit

            for g in range(TT // 2):
                pieces.append(mk_mm(ch, g))
            pieces.append(mk_tail(ch))
        return pieces

    def emit_normalize(b, sums, hs, he, chs=(0,)):
        # bounce the per-head sum rows through DRAM so one DMA can land
        # them on partitions 0..G-1 (SBUF DMA dst needs 32-aligned start),
        # then one batched reciprocal per head-group/chunk (DVE recip cost
        # is free-size-bound, so the partitions come for free)
        G = he - hs
        for ch in chs:
            cs = slice(ch * CS, (ch + 1) * CS) if len(chs) > 1 else slice(0, N)
            W = cs.stop - cs.start
            sums_sb = nrm.tile([H, N], BF16, tag="sums_sb", name="sums_sb")
            nc.sync.dma_start(sums_sb[:G, :W], sums[hs:he, cs])
            rsum = nrm.tile([H, N], F32, tag="rsum", name="rsum")
            nc.vector.reciprocal(rsum[:G, :W], sums_sb[:G, :W])
            rdram = dram.tile([H, N], F32, tag="rdram", name="rdram")
            nc.sync.dma_start(rdram[:G, :W], rsum[:G, :W])
            for hh in range(G):
                h = hs + hh
                ft, pr = h // 2, (h % 2) * 64
                rb = nrm2.tile([P, N], F32, tag="rb", name="rb")
                nc.gpsimd.dma_start(
                    rb[pr : pr + 64, :W],
                    rdram[hh, cs.start : cs.stop].partition_broadcast(64),
                )
                dst = OT[b][pr : pr + 64, ft, cs]
                nc.vector.tensor_tensor(
                    dst, dst, rb[pr : pr + 64, :W], OP.mult
                )

    def emit_head(b, h, pv_pieces, pop_filler, n_pops, sums, forced=()):
        ft, pr = h // 2, (h % 2) * 64
        Pt = pt_pool.tile([P, TT, N], BF16, tag="Pt", name="Pt")
        npv = len(pv_pieces)
        forced = list(forced)
        nfo = len(forced)
        pi = 0
        popped = 0
        for kt in range(TT):
            ps = sa_tile()
            for ch in range(NCH):
                nc.tensor.matmul(
                    ps[:, ch * CS : (ch + 1) * CS],
                    qkT[b][pr : pr + 64, KO + ft, kt * P : (kt + 1) * P],
                    qkT[b][pr : pr + 64, ft, ch * CS : (ch + 1) * CS],
                    start=True,
                    stop=True,
                )
            nc.scalar.activation(Pt[:, kt, :], ps[:, :N], AF.Exp, scale=0.125)
            quota = ((kt + 1) * npv) // TT - (kt * npv) // TT
            for _ in range(quota):
                pv_pieces[pi]()
                pi += 1
            fq = ((kt + 1) * nfo) // TT - (kt * nfo) // TT
            for _ in range(fq):
                forced.pop(0)()
            pq = ((kt + 1) * n_pops) // TT - (kt * n_pops) // TT
            for _ in range(pq):
                pop_filler()
                popped += 1
        while pi < len(pv_pieces):
            pv_pieces[pi]()
            pi += 1
        return Pt

    def emit_attention(b, fillers, pops_per_head, sums, pre=None, forced={}):
        fi = [0]

        def pop_filler():
            if fi[0] < len(fillers):
                fillers[fi[0]]()
                fi[0] += 1

        h0, Pt0 = pre
        pieces = make_pv_pieces(b, h0, Pt0, sums)
        for h in range(h0 + 1, H):
            Pt = emit_head(
                b,
                h,
                pieces,
                pop_filler,
                pops_per_head.get(h, 0),
                sums,
                forced=forced.get(h, ()),
            )
            pieces = make_pv_pieces(b, h, Pt, sums)
            if h == 6:
                emit_normalize(b, sums, 0, 6)
            elif h == 10:
                emit_normalize(b, sums, 6, 10)
        for p in pieces:
            p()
        # drain unused fillers
        while fi[0] < len(fillers):
            fillers[fi[0]]()
            fi[0] += 1
        emit_normalize(b, sums, 10, H)

    # ---------------- top-level schedule ------------------------------
    def prep_chunks(b):
        out = [xprep_chunk(b, tt) for tt in range(TT)]
        out += [qk_chunk(b, m, ch) for m in range(2 * KO) for ch in range(NCH)]
        return out

    def vproj_chunks(b):
        return [v_chunk(b, tt, oc) for tt in range(TT) for oc in range(2)]

    def noop():
        pass

    # Startup: interleave x(0) prep with the weight loads so the PE has
    # transpose work while the big weight DMAs stream in.
    wchunks_qk = [wqkv_chunk(ko, 0) for ko in range(KO)] + [
        wqkv_chunk(ko, 1) for ko in range(KO)
    ]
    wchunks_late = [wqkv_chunk(ko, 2) for ko in range(KO)] + [
        wproj_chunk(ko) for ko in range(KO)
    ]
    for tt in range(TT):
        xprep_chunk(0, tt)()
        if tt == 0:
            bias_chunk()
    for c in wchunks_qk:
        c()
    # QK for head-pairs 0-2 upfront (enough to start attention); pairs
    # 3-5 become early fillers inside batch-0 attention
    qkc = [
        qk_chunk(0, m, ch)
        for ft in range(KO // 2)
        for m in (ft, KO + ft)
        for ch in range(NCH)
    ]
    qk_rest = [
        qk_chunk(0, m, ch)
        for ft in range(KO // 2, KO)
        for m in (ft, KO + ft)
        for ch in range(NCH)
    ]
    li = 0
    for i, c in enumerate(qkc + qk_rest):
        c()
        if li < len(wchunks_late):
            wchunks_late[li]()
            li += 1
    while li < len(wchunks_late):
        wchunks_late[li]()
        li += 1

    OT[0] = ot_pool.tile([P, KO, N], BF16, tag="OT", name="OT")
    sums0 = dram.tile([H, N], BF16, tag="sums", name="sums")
    sums_of = [sums0] + [None] * (n_batch - 1)
    pre_of = [(0, emit_scores(0, 0, noop, 0))] + [None] * (n_batch - 1)
    for c in vproj_chunks(0):
        c()

    for b in range(n_batch):
        fillers = prep_chunks(b + 1) if b + 1 < n_batch else []
        forced = {}
        if b > 0:
            fillers = fillers + [
                oproj_chunk(b - 1, tt, oc) for tt in range(TT) for oc in range(2)
            ]
        # spread fillers over heads; delay pops on later batches so filler
        # dependencies (previous batch's normalize) are ready
        pops = {}
        nf = len(fillers)
        start_h = (1 if b == 0 else 3)
        nh = H - start_h
        for i, h in enumerate(range(start_h, H)):
            share = (nf * (i + 1)) // nh - (nf * i) // nh
            if share:
                pops[h] = share
        emit_attention(b, fillers, pops, sums_of[b], pre=pre_of[b], forced=forced)
        if b + 1 < n_batch:
            # next batch: pre-score head 0 right away (ACT runs it during
            # the V-projection bridge), then the V bridge
            OT[b + 1] = ot_pool.tile([P, KO, N], BF16, tag="OT", name="OT")
            sums_of[b + 1] = dram.tile([H, N], BF16, tag="sums", name="sums")
            pre_of[b + 1] = (0, emit_scores(b + 1, 0, noop, 0))
            for c in vproj_chunks(b + 1):
                c()
    for tt in range(TT):
        for oc in range(2):
            oproj_chunk(n_batch - 1, tt, oc)()




def build_graph(n_batch=BPC, N=N_FULL, n_cores=N_CORES):
    nc = bacc.Bacc(
        "TRN2", target_bir_lowering=False, debug=False, num_devices=n_cores
    )
    x_d = nc.dram_tensor("x", [n_batch, N, E], F32, kind="ExternalInput").ap()
    wqkv_d = nc.dram_tensor("w_qkv", [E, 3 * E], F32, kind="ExternalInput").ap()
    bqkv_d = nc.dram_tensor("b_qkv", [3 * E], F32, kind="ExternalInput").ap()
    wproj_d = nc.dram_tensor("w_proj", [E, E], F32, kind="ExternalInput").ap()
    bproj_d = nc.dram_tensor("b_proj", [E], F32, kind="ExternalInput").ap()
    out_d = nc.dram_tensor("out", [n_batch, N, E], F32, kind="ExternalOutput").ap()

    with tile.TileContext(nc) as tc, ExitStack() as ctx:
        _body(ctx, tc, out_d, x_d, wqkv_d, bqkv_d, wproj_d, bproj_d, n_batch, N)
    nc.compile()
    return nc


_NC_CACHE = {}


def _get_graph():
    if "nc" not in _NC_CACHE:
        _NC_CACHE["nc"] = build_graph()
    return _NC_CACHE["nc"]


def run_on_hw(x, w_qkv, b_qkv, w_proj, b_proj, trace=False):
    nc = _get_graph()
    x = np.ascontiguousarray(np.asarray(x, dtype=np.float32))
    shared = {
        "w_qkv": np.ascontiguousarray(np.asarray(w_qkv, dtype=np.float32)),
        "b_qkv": np.ascontiguousarray(np.asarray(b_qkv, dtype=np.float32)),
        "w_proj": np.ascontiguousarray(np.asarray(w_proj, dtype=np.float32)),
        "b_proj": np.ascontiguousarray(np.asarray(b_proj, dtype=np.float32)),
    }
    in_maps = [
        {"x": x[i * BPC : (i + 1) * BPC], **shared} for i in range(N_CORES)
    ]
    res = run_bass_kernel_spmd(
        nc, in_maps, core_ids=list(range(N_CORES)), trace=trace
    )
    out = np.concatenate([r["out"] for r in res.results], axis=0)
    return out, res


def kernel(x, w_qkv, b_qkv, w_proj, b_proj):
    out, _ = run_on_hw(x, w_qkv, b_qkv, w_proj, b_proj)
    return out


# revision 34
# speedup vs baseline: 1.0328x; 1.0328x over previous
"""Multi-head attention (B=16, N=1024, E=768, H=12, D=64) on 8 TRN2 NeuronCores.

Strategy: data-parallel over batch (2 batches per core, no collectives).
Per-core kernel computes qkv = x @ w_qkv + b, per-head attention, out proj.

Layout choices (all compute in bf16, fp32 PSUM accumulation):
  - x^T (embed-on-partitions) built once per batch via PE transpose.
  - q^T/k^T produced feature-major [feat, tokens]: S^T = k^T.T @ q^T needs
    no transposes at all (keys on PSUM partitions, queries on free dim).
  - exp fused into the PSUM->SBUF copy on the scalar (ACT) engine; softmax
    denominator obtained free by appending an all-ones column to each
    v-tile in the P@V matmul (row 64 of the PV psum = sum of exp scores).
  - normalization: fast approx reciprocal of the sum row, gpsimd partition
    broadcast, one DVE multiply; odd heads partition-shifted into the
    emb-major O^T store via a small SBUF->SBUF DMA.
  - out = O^T.T @ w_proj directly token-major for a contiguous DRAM store.

Scheduling: attention is ACT(exp)-bound per head, so PE work from other
phases is interleaved as "fillers" — batch b+1's x^T/QKV projections run
inside batch b's attention, batch b-1's out-projection inside batch b's —
keeping TensorE dense (HAM stays un-throttled) while the scalar engine
streams the exps.
"""

from contextlib import ExitStack

import numpy as np

import concourse.bass as bass
import concourse.mybir as mybir
import concourse.tile as tile
from concourse import bacc
from concourse.bass_utils import run_bass_kernel_spmd
from concourse.masks import make_identity

F32 = mybir.dt.float32
BF16 = mybir.dt.bfloat16
AF = mybir.ActivationFunctionType
OP = mybir.AluOpType

P = 128
E = 768          # embed dim
H = 12           # heads
D = 64           # head dim
KO = E // P      # 6 contraction subtiles over embed
B_FULL = 16
N_FULL = 1024
N_CORES = 8
BPC = B_FULL // N_CORES  # batches per core


def _body(ctx, tc, out_d, x_d, wqkv_d, bqkv_d, wproj_d, bproj_d, n_batch, N):
    nc = tc.nc
    TT = N // P                 # token tiles per batch
    CS = min(512, N)            # query-chunk size
    NCH = N // CS               # query chunks

    const = ctx.enter_context(tc.tile_pool(name="const", bufs=1))
    xt_pool = ctx.enter_context(tc.tile_pool(name="xt", bufs=1))
    qk_pool = ctx.enter_context(tc.tile_pool(name="qk", bufs=2))
    v_pool = ctx.enter_context(tc.tile_pool(name="v", bufs=1))
    pt_pool = ctx.enter_context(tc.tile_pool(name="pt", bufs=2))
    ot_pool = ctx.enter_context(tc.tile_pool(name="ot", bufs=1))
    small = ctx.enter_context(tc.tile_pool(name="small", bufs=3))
    wstage = ctx.enter_context(tc.tile_pool(name="wstage", bufs=3))
    nrm = ctx.enter_context(tc.tile_pool(name="nrm", bufs=1))
    nrm2 = ctx.enter_context(tc.tile_pool(name="nrm2", bufs=2))
    dram = ctx.enter_context(tc.tile_pool(name="dram", bufs=1, space="DRAM"))
    # [128, 768] f32 staging shared by weight loads, x loads, and out tiles
    osb_pool = ctx.enter_context(tc.tile_pool(name="osb", bufs=3))
    psA = ctx.enter_context(tc.tile_pool(name="psA", bufs=2, space="PSUM"))
    psB = ctx.enter_context(tc.tile_pool(name="psB", bufs=3, space="PSUM"))
    psO = ctx.enter_context(tc.tile_pool(name="psO", bufs=1, space="PSUM"))

    def sa_tile():
        return psA.tile([P, 1024], F32, tag="psa", name="psa")

    def sb_tile():
        return psB.tile([P, 512], F32, tag="psb", name="psb")

    def stage_tile():
        return osb_pool.tile([P, E], F32, tag="osb", name="osb")

    # ---------------- constants ----------------
    identity = const.tile([P, P], F32, tag="ident")
    make_identity(nc, identity)

    wqkv_sb = const.tile([P, KO, 3 * E], BF16, tag="wqkv")
    wproj_sb = const.tile([P, KO, E], BF16, tag="wproj")
    bqk_sb = const.tile([P, 2 * KO], F32, tag="bqk")
    bv_sb = const.tile([P, E], F32, tag="bv")
    bp_sb = const.tile([P, E], F32, tag="bp")
    wq_r = wqkv_d.rearrange("(ko p) (c n) -> p ko c n", p=P, n=E)
    wp_r = wproj_d.rearrange("(ko p) n -> p ko n", p=P)

    def bias_chunk():
        # q/k bias as per-partition columns: feature f = m*128+p -> [128,12]
        nc.sync.dma_start(
            bqk_sb[:], bqkv_d[: 2 * E].rearrange("(m p) -> p m", p=P)
        )
        # v / proj bias replicated across partitions (free-dim varying)
        nc.gpsimd.dma_start(
            bv_sb[:], bqkv_d[2 * E : 3 * E].partition_broadcast(P)
        )
        nc.gpsimd.dma_start(bp_sb[:], bproj_d.partition_broadcast(P))

    def wqkv_chunk(ko, c):
        def emit():
            t = wstage.tile([P, E], F32, tag="ws", name="ws")
            nc.sync.dma_start(t[:], wq_r[:, ko, c, :])
            nc.vector.tensor_copy(wqkv_sb[:, ko, c * E : (c + 1) * E], t[:])

        return emit

    def wproj_chunk(ko):
        def emit():
            t = wstage.tile([P, E], F32, tag="ws", name="ws")
            nc.sync.dma_start(t[:], wp_r[:, ko, :])
            nc.vector.tensor_copy(wproj_sb[:, ko, :], t[:])

        return emit

    # Per-batch state handles
    xT = [None] * n_batch
    qkT = [None] * n_batch
    v_sb = [None] * n_batch
    OT = [None] * n_batch

    # ---------------- phase emitters (closures) -----------------------
    def xprep_chunk(b, tt):
        def emit():
            if tt == 0:
                xT[b] = xt_pool.tile([P, KO, N], BF16, tag="xT", name="xT")
            xtmp = stage_tile()
            nc.sync.dma_start(xtmp[:], x_d[b, tt * P : (tt + 1) * P, :])
            for ko in range(KO):
                pst = sb_tile()
                nc.tensor.transpose(
                    pst[:, :P], xtmp[:, ko * P : (ko + 1) * P], identity
                )
                nc.vector.tensor_copy(
                    xT[b][:, ko, tt * P : (tt + 1) * P], pst[:, :P]
                )

        return emit

    def qk_chunk(b, m, ch):
        def emit():
            if m == 0 and ch == 0:
                qkT[b] = qk_pool.tile([P, 2 * KO, N], BF16, tag="qkT", name="qkT")
            ps = sb_tile()
            for ko in range(KO):
                nc.tensor.matmul(
                    ps[:, :CS],
                    wqkv_sb[:, ko, m * P : (m + 1) * P],
                    xT[b][:, ko, ch * CS : (ch + 1) * CS],
                    start=(ko == 0),
                    stop=(ko == KO - 1),
                )
            nc.vector.tensor_tensor(
                qkT[b][:, m, ch * CS : (ch + 1) * CS],
                ps[:, :CS],
                bqk_sb[:, m : m + 1].to_broadcast([P, CS]),
                OP.add,
            )

        return emit

    def v_chunk(b, tt, oc):
        def emit():
            if tt == 0 and oc == 0:
                v_sb[b] = v_pool.tile([P, TT, H * 65], BF16, tag="v", name="v")
                ones_cols = v_sb[b].rearrange("p t (h c) -> p t h c", c=65)[
                    :, :, :, 64
                ]
                nc.vector.memset(ones_cols, 1.0)
            ocs = 512 if oc == 0 else 256
            nh = 8 if oc == 0 else 4
            ps = sb_tile()
            for ko in range(KO):
                nc.tensor.matmul(
                    ps[:, :ocs],
                    xT[b][:, ko, tt * P : (tt + 1) * P],
                    wqkv_sb[:, ko, 2 * E + oc * 512 : 2 * E + oc * 512 + ocs],
                    start=(ko == 0),
                    stop=(ko == KO - 1),
                )
            for hh in range(nh):
                h = oc * 8 + hh
                nc.vector.tensor_tensor(
                    v_sb[b][:, tt, h * 65 : h * 65 + 64],
                    ps[:, hh * 64 : (hh + 1) * 64],
                    bv_sb[:, h * 64 : (h + 1) * 64],
                    OP.add,
                )

        return emit

    _osb_state = {}

    def oproj_chunk(b, tt, oc):
        def emit():
            if oc == 0:
                _osb_state[(b, tt)] = stage_tile()
            osb = _osb_state[(b, tt)]
            ocs = 512 if oc == 0 else 256
            ps = sb_tile()
            for ko in range(KO):
                nc.tensor.matmul(
                    ps[:, :ocs],
                    OT[b][:, ko, tt * P : (tt + 1) * P],
                    wproj_sb[:, ko, oc * 512 : oc * 512 + ocs],
                    start=(ko == 0),
                    stop=(ko == KO - 1),
                )
            nc.vector.tensor_tensor(
                osb[:, oc * 512 : oc * 512 + ocs],
                ps[:, :ocs],
                bp_sb[:, oc * 512 : oc * 512 + ocs],
                OP.add,
            )
            if oc == 1:
                nc.sync.dma_start(
                    out_d[b, tt * P : (tt + 1) * P, :], osb[:]
                )

        return emit

    # ---------------- attention -----------------------
    def emit_scores(b, h, pop_filler, n_pops):
        ft, pr = h // 2, (h % 2) * 64
        pop_at = {3: 1, 7: 1} if n_pops <= 2 else {1: 1, 3: 1, 5: 1, 7: n_pops - 3}
        if n_pops == 1:
            pop_at = {7: 1}
        if n_pops == 0:
            pop_at = {}
        Pt = pt_pool.tile([P, TT, N], BF16, tag="Pt", name="Pt")
        for kt in range(TT):
            ps = sa_tile()
            for ch in range(NCH):
                nc.tensor.matmul(
                    ps[:, ch * CS : (ch + 1) * CS],
                    qkT[b][pr : pr + 64, KO + ft, kt * P : (kt + 1) * P],
                    qkT[b][pr : pr + 64, ft, ch * CS : (ch + 1) * CS],
                    start=True,
                    stop=True,
                )
            nc.scalar.activation(Pt[:, kt, :], ps[:, :N], AF.Exp, scale=0.125)
            for _ in range(pop_at.get(kt, 0)):
                pop_filler()
        return Pt

    def make_pv_pieces(b, h, Pt, sums):
        """PV matmuls + epilogue for head h as a list of closures, to be
        consumed interleaved with the NEXT head's score matmuls so the
        scalar engine never starves while PE runs the PV block."""
        ft, pr = h // 2, (h % 2) * 64
        pieces = []
        state = {}
        for ch in range(NCH):

            def mk_mm(ch, g):
                def emit():
                    if g == 0:
                        state[ch] = psO.tile([P, 512], F32, tag="po", name="po")
                    po = state[ch]
                    for kt in (2 * g, 2 * g + 1):
                        nc.tensor.matmul(
                            po[:65, :CS],
                            v_sb[b][:, kt, h * 65 : (h + 1) * 65],
                            Pt[:, kt, ch * CS : (ch + 1) * CS],
                            start=(kt == 0),
                            stop=(kt == TT - 1),
                        )

                return emit

            def mk_tail(ch):
                def emit():
                    po = state[ch]
                    otmp = small.tile([65, 512], BF16, tag="otmp", name="otmp")
                    nc.vector.tensor_copy(otmp[:, :CS], po[0:65, :CS])
                    nc.sync.dma_start(
                        sums[h, ch * CS : (ch + 1) * CS], otmp[64:65, :CS]
                    )
                    nc.sync.dma_start(
                        OT[b][pr : pr + 64, ft, ch * CS : (ch + 1) * CS],
                        otmp[0:64, :CS],
                    )

                return emit

            for g in range(TT // 2):
                pieces.append(mk_mm(ch, g))
            pieces.append(mk_tail(ch))
        return pieces

    def emit_normalize(b, sums, hs, he, chs=(0,)):
        # bounce the per-head sum rows through DRAM so one DMA can land
        # them on partitions 0..G-1 (SBUF DMA dst needs 32-aligned start),
        # then one batched reciprocal per head-group/chunk (DVE recip cost
        # is free-size-bound, so the partitions come for free)
        G = he - hs
        for ch in chs:
            cs = slice(ch * CS, (ch + 1) * CS) if len(chs) > 1 else slice(0, N)
            W = cs.stop - cs.start
            sums_sb = nrm.tile([H, N], BF16, tag="sums_sb", name="sums_sb")
            nc.sync.dma_start(sums_sb[:G, :W], sums[hs:he, cs])
            rsum = nrm.tile([H, N], F32, tag="rsum", name="rsum")
            nc.vector.reciprocal(rsum[:G, :W], sums_sb[:G, :W])
            rdram = dram.tile([H, N], F32, tag="rdram", name="rdram")
            nc.sync.dma_start(rdram[:G, :W], rsum[:G, :W])
            for hh in range(G):
                h = hs + hh
                ft, pr = h // 2, (h % 2) * 64
                rb = nrm2.tile([P, N], F32, tag="rb", name="rb")
                nc.gpsimd.dma_start(
                    rb[pr : pr + 64, :W],
                    rdram[hh, cs.start : cs.stop].partition_broadcast(64),
                )
                dst = OT[b][pr : pr + 64, ft, cs]
                nc.vector.tensor_tensor(
                    dst, dst, rb[pr : pr + 64, :W], OP.mult
                )

    def emit_head(b, h, pv_pieces, pop_filler, n_pops, sums, forced=()):
        ft, pr = h // 2, (h % 2) * 64
        Pt = pt_pool.tile([P, TT, N], BF16, tag="Pt", name="Pt")
        npv = len(pv_pieces)
        forced = list(forced)
        nfo = len(forced)
        pi = 0
        popped = 0
        for kt in range(TT):
            ps = sa_tile()
            for ch in range(NCH):
                nc.tensor.matmul(
                    ps[:, ch * CS : (ch + 1) * CS],
                    qkT[b][pr : pr + 64, KO + ft, kt * P : (kt + 1) * P],
                    qkT[b][pr : pr + 64, ft, ch * CS : (ch + 1) * CS],
                    start=True,
                    stop=True,
                )
            nc.scalar.activation(Pt[:, kt, :], ps[:, :N], AF.Exp, scale=0.125)
            quota = ((kt + 1) * npv) // TT - (kt * npv) // TT
            for _ in range(quota):
                pv_pieces[pi]()
                pi += 1
            fq = ((kt + 1) * nfo) // TT - (kt * nfo) // TT
            for _ in range(fq):
                forced.pop(0)()
            pq = ((kt + 1) * n_pops) // TT - (kt * n_pops) // TT
            for _ in range(pq):
                pop_filler()
                popped += 1
        while pi < len(pv_pieces):
            pv_pieces[pi]()
            pi += 1
        return Pt

    def emit_attention(b, fillers, pops_per_head, sums, pre=None, forced={}):
        fi = [0]

        def pop_filler():
            if fi[0] < len(fillers):
                fillers[fi[0]]()
                fi[0] += 1

        h0, Pt0 = pre
        pieces = make_pv_pieces(b, h0, Pt0, sums)
        for h in range(h0 + 1, H):
            Pt = emit_head(
                b,
                h,
                pieces,
                pop_filler,
                pops_per_head.get(h, 0),
                sums,
                forced=forced.get(h, ()),
            )
            pieces = make_pv_pieces(b, h, Pt, sums)
            if h == 6:
                emit_normalize(b, sums, 0, 6)
            elif h == 10:
                emit_normalize(b, sums, 6, 10)
        for p in pieces:
            p()
        # drain unused fillers
        while fi[0] < len(fillers):
            fillers[fi[0]]()
            fi[0] += 1
        emit_normalize(b, sums, 10, H)

    # ---------------- top-level schedule ------------------------------
    def prep_chunks(b):
        out = [xprep_chunk(b, tt) for tt in range(TT)]
        out += [qk_chunk(b, m, ch) for m in range(2 * KO) for ch in range(NCH)]
        return out

    def vproj_chunks(b):
        return [v_chunk(b, tt, oc) for tt in range(TT) for oc in range(2)]

    def noop():
        pass

    # Startup: interleave x(0) prep with the weight loads so the PE has
    # transpose work while the big weight DMAs stream in.
    wchunks_qk = [wqkv_chunk(ko, 0) for ko in range(KO)] + [
        wqkv_chunk(ko, 1) for ko in range(KO)
    ]
    wchunks_late = [wqkv_chunk(ko, 2) for ko in range(KO)] + [
        wproj_chunk(ko) for ko in range(KO)
    ]
    for tt in range(TT):
        xprep_chunk(0, tt)()
        if tt == 0:
            bias_chunk()
    for c in wchunks_qk:
        c()
    # QK for head-pairs 0-2 upfront (enough to start attention); pairs
    # 3-5 become early fillers inside batch-0 attention
    qkc = [
        qk_chunk(0, m, ch)
        for ft in range(KO // 2)
        for m in (ft, KO + ft)
        for ch in range(NCH)
    ]
    qk_rest = [
        qk_chunk(0, m, ch)
        for ft in range(KO // 2, KO)
        for m in (ft, KO + ft)
        for ch in range(NCH)
    ]
    li = 0
    for i, c in enumerate(qkc + qk_rest):
        c()
        if li < len(wchunks_late):
            wchunks_late[li]()
            li += 1
    while li < len(wchunks_late):
        wchunks_late[li]()
        li += 1

    OT[0] = ot_pool.tile([P, KO, N], BF16, tag="OT", name="OT")
    sums0 = dram.tile([H, N], BF16, tag="sums", name="sums")
    sums_of = [sums0] + [None] * (n_batch - 1)
    pre_of = [(0, emit_scores(0, 0, noop, 0))] + [None] * (n_batch - 1)
    for c in vproj_chunks(0):
        c()

    for b in range(n_batch):
        fillers = prep_chunks(b + 1) if b + 1 < n_batch else []
        forced = {}
        if b > 0:
            fillers = fillers + [
                oproj_chunk(b - 1, tt, oc) for tt in range(TT) for oc in range(2)
            ]
        # spread fillers over heads; delay pops on later batches so filler
        # dependencies (previous batch's normalize) are ready
        pops = {}
        nf = len(fillers)
        start_h = (1 if b == 0 else 3)
        nh = H - start_h
        for i, h in enumerate(range(start_h, H)):
            share = (nf * (i + 1)) // nh - (nf * i) // nh
            if share:
                pops[h] = share
        emit_attention(b, fillers, pops, sums_of[b], pre=pre_of[b], forced=forced)
        if b + 1 < n_batch:
            # next batch: pre-score head 0 right away (ACT runs it during
            # the V-projection bridge), then the V bridge
            OT[b + 1] = ot_pool.tile([P, KO, N], BF16, tag="OT", name="OT")
            sums_of[b + 1] = dram.tile([H, N], BF16, tag="sums", name="sums")
            pre_of[b + 1] = (0, emit_scores(b + 1, 0, noop, 0))
            for c in vproj_chunks(b + 1):
                c()
    for tt in range(TT):
        for oc in range(2):
            oproj_chunk(n_batch - 1, tt, oc)()




def build_graph(n_batch=BPC, N=N_FULL, n_cores=N_CORES):
    nc = bacc.Bacc(
        "TRN2", target_bir_lowering=False, debug=False, num_devices=n_cores
    )
    x_d = nc.dram_tensor("x", [n_batch, N, E], F32, kind="ExternalInput").ap()
    wqkv_d = nc.dram_tensor("w_qkv", [E, 3 * E], F32, kind="ExternalInput").ap()
    bqkv_d = nc.dram_tensor("b_qkv", [3 * E], F32, kind="ExternalInput").ap()
    wproj_d = nc.dram_tensor("w_proj", [E, E], F32, kind="ExternalInput").ap()
    bproj_d = nc.dram_tensor("b_proj", [E], F32, kind="ExternalInput").ap()
    out_d = nc.dram_tensor("out", [n_batch, N, E], F32, kind="ExternalOutput").ap()

    with tile.TileContext(nc) as tc, ExitStack() as ctx:
        _body(ctx, tc, out_d, x_d, wqkv_d, bqkv_d, wproj_d, bproj_d, n_batch, N)
    nc.compile()
    return nc


_NC_CACHE = {}


def _get_graph():
    if "nc" not in _NC_CACHE:
        _NC_CACHE["nc"] = build_graph()
    return _NC_CACHE["nc"]


def run_on_hw(x, w_qkv, b_qkv, w_proj, b_proj, trace=False):
    nc = _get_graph()
    x = np.ascontiguousarray(np.asarray(x, dtype=np.float32))
    shared = {
        "w_qkv": np.ascontiguousarray(np.asarray(w_qkv, dtype=np.float32)),
        "b_qkv": np.ascontiguousarray(np.asarray(b_qkv, dtype=np.float32)),
        "w_proj": np.ascontiguousarray(np.asarray(w_proj, dtype=np.float32)),
        "b_proj": np.ascontiguousarray(np.asarray(b_proj, dtype=np.float32)),
    }
    in_maps = [
        {"x": x[i * BPC : (i + 1) * BPC], **shared} for i in range(N_CORES)
    ]
    res = run_bass_kernel_spmd(
        nc, in_maps, core_ids=list(range(N_CORES)), trace=trace
    )
    out = np.concatenate([r["out"] for r in res.results], axis=0)
    return out, res


def kernel(x, w_qkv, b_qkv, w_proj, b_proj):
    out, _ = run_on_hw(x, w_qkv, b_qkv, w_proj, b_proj)
    return out
